# revision 10
# baseline (speedup 1.0000x reference)
"""KAN layer (piecewise-linear spline lookup) on 8 TRN2 NeuronCores.

Math: the reference computes, per (batch b, feature f):
    u   = (x + 3) / h              with h = 6/63  (uniform grid, 64 knots)
    i   = clip(searchsorted(g, x), 1, 63)
    y  += (1-t) * C[f, i-1, :] + t * C[f, i, :],   t = u - (i-1)
This is, for each (f, o), a continuous piecewise-linear function of u with
knots at u = 1..62 and linear extrapolation beyond — exactly representable
in the ReLU basis:
    y[b, o] = sum_f  alpha[f,o] + beta[f,o] * u[f,b]
                   + sum_{k=1}^{62} gamma[f,k,o] * relu(u[f,b] - k)
with
    alpha = C[:, 0, :]
    beta  = C[:, 1, :] - C[:, 0, :]            (first slope)
    gamma[:, k, :] = C[:,k+1,:] - 2*C[:,k,:] + C[:,k-1,:]   (slope changes)
This removes the gather entirely: the device computes 62 cheap fused
relu-shift tiles R_k = relu(uT - k) on DVE/ACT and contracts everything
with 64 accumulating TensorE matmuls (K=128 features on partitions).

Sharding: data-parallel — batch 4096 split into 8 shards of 512; the small
coeff table and bias are replicated to every core (per the sharding hint).
Host does layout-only prep (transpose/reshape/slice); all value-producing
compute runs on device.
"""

import numpy as np

import concourse.bass as bass
import concourse.mybir as mybir
import concourse.tile as tile
from concourse import bacc
from concourse.bass_utils import run_bass_kernel_spmd

F32 = mybir.dt.float32
ALU = mybir.AluOpType
ACTF = mybir.ActivationFunctionType

IN_DIM = 128     # features (partition dim)
OUT_DIM = 64
GRID = 64
B = 4096
N_CORES = 8
BS = B // N_CORES          # 512 batch rows per core
X_MIN, X_MAX = -3.0, 3.0
H = (X_MAX - X_MIN) / (GRID - 1)          # 6/63
INV_H = 1.0 / H
U_OFF = -X_MIN / H                         # +3/h
NK = GRID - 2                              # 62 interior knots, k = 1..62
N_WARM = 16                                # PE HAM warmup matmuls

# s-chunking: s_k = C[:,k+1,:] - C[:,k,:] for k = 0..62 (63 slopes), in 8
# overlapping chunks so gamma chunk c (k = 8c+1 .. min(8c+8, 62)) only needs
# s-chunk c. Chunk c holds s_{8c} .. s_{min(8c+8, 62)}.
def _chunks():
    out = []
    for c in range(8):
        s_lo = 8 * c
        s_hi = min(8 * c + 8, 62)          # inclusive
        g_lo = 8 * c + 1
        g_hi = min(8 * c + 8, 62)          # inclusive
        out.append((s_lo, s_hi, g_lo, g_hi))
    return out


def _dve_k(k: int) -> bool:
    # ~2/5 of relu tiles on DVE (it also owns the gamma prep), rest on ACT
    return (k % 5) in (0, 3)


def build_program(reps: int = 1):
    nc = bacc.Bacc(
        "TRN2",
        target_bir_lowering=False,
        debug=False,
        num_devices=N_CORES,
    )
    xT_d = nc.dram_tensor("xT", [IN_DIM, BS], F32, kind="ExternalInput")
    coeff_d = nc.dram_tensor("coeff", [IN_DIM, GRID * OUT_DIM], F32, kind="ExternalInput")
    bias_d = nc.dram_tensor("bias", [OUT_DIM, 1], F32, kind="ExternalInput")
    # device constant: column k-1 holds -k (ACT Relu bias per knot)
    negk_d = nc.dram_tensor("negk", [IN_DIM, NK], F32, kind="ExternalInput")
    yT_d = nc.dram_tensor("yT", [OUT_DIM, BS], F32, kind="ExternalOutput")

    with tile.TileContext(nc) as tc:
        for rep in range(reps):
            _emit(tc, yT_d.ap(), xT_d.ap(), coeff_d.ap(), bias_d.ap(), negk_d.ap(),
                  warmup=(rep == 0))

    nc.compile()
    return nc


def _emit(tc, yT, xT, coeffR, biasd, negkd, warmup=True):
    nc = tc.nc
    ck = _chunks()

    with (
        tc.tile_pool(name="const", bufs=1) as cpool,
        tc.tile_pool(name="sg", bufs=1) as sgpool,
        tc.tile_pool(name="rt", bufs=8) as rpool,
        tc.tile_pool(name="py", bufs=1, space="PSUM") as ppool,
        tc.tile_pool(name="pw", bufs=1, space="PSUM") as wpool,
    ):
        # ---- constants / inputs ----
        C = cpool.tile([IN_DIM, GRID * OUT_DIM], F32, tag="C")
        # split the 2 MB coeff load so early knots arrive first
        n_dma = 4
        cw = GRID * OUT_DIM // n_dma
        for d in range(n_dma):
            nc.sync.dma_start(
                out=C[:, d * cw : (d + 1) * cw], in_=coeffR[:, d * cw : (d + 1) * cw]
            )
        xt = cpool.tile([IN_DIM, BS], F32, tag="xt")
        nc.sync.dma_start(out=xt[:], in_=xT[:, :])
        bt = cpool.tile([OUT_DIM, 1], F32, tag="bt")
        nc.sync.dma_start(out=bt[:], in_=biasd[:, :])
        nk = cpool.tile([IN_DIM, NK], F32, tag="nk")
        nc.sync.dma_start(out=nk[:], in_=negkd[:, :])

        ones = cpool.tile([IN_DIM, BS], F32, tag="ones")
        nc.vector.memset(ones[:], 1.0)

        u = cpool.tile([IN_DIM, BS], F32, tag="u")
        nc.vector.tensor_scalar(u[:], xt[:], INV_H, U_OFF, ALU.mult, ALU.add)

        # ---- PE warmup during the coeff DMA (HAM clock-gate) ----
        if warmup:
            warm = wpool.tile([OUT_DIM, BS], F32, tag="warm")
            for _ in range(N_WARM):
                nc.tensor.matmul(
                    warm[:], ones[:, :OUT_DIM], ones[:], start=True, stop=True
                )

        yp = ppool.tile([OUT_DIM, BS], F32, tag="yp")

        # alpha term: sum_f C[f, 0, :]  -> lhsT = C[:, 0:64], rhs = ones
        nc.tensor.matmul(yp[:], C[:, 0:OUT_DIM], ones[:], start=True, stop=False)

        s_tiles = {}
        g_tiles = {}

        def make_chunk(c):
            s_lo, s_hi, g_lo, g_hi = ck[c]
            ns = s_hi - s_lo + 1
            st = sgpool.tile([IN_DIM, ns * OUT_DIM], F32, tag=f"s{c}")
            nc.vector.tensor_tensor(
                out=st[:],
                in0=C[:, (s_lo + 1) * OUT_DIM : (s_hi + 2) * OUT_DIM],
                in1=C[:, s_lo * OUT_DIM : (s_hi + 1) * OUT_DIM],
                op=ALU.subtract,
            )
            s_tiles[c] = st
            ng = g_hi - g_lo + 1
            gt = sgpool.tile([IN_DIM, ng * OUT_DIM], F32, tag=f"g{c}")
            nc.vector.tensor_tensor(
                out=gt[:],
                in0=st[:, OUT_DIM : (ng + 1) * OUT_DIM],
                in1=st[:, 0 : ng * OUT_DIM],
                op=ALU.subtract,
            )
            g_tiles[c] = gt

        # beta term needs s_0 (chunk 0)
        make_chunk(0)
        nc.tensor.matmul(yp[:], s_tiles[0][:, 0:OUT_DIM], u[:], start=False, stop=False)

        # ---- main loop: R_k = relu(u - k), accumulate gamma_k^T @ R_k ----
        for k in range(1, NK + 1):
            c = (k - 1) // 8
            if c not in g_tiles:
                make_chunk(c)
            g_lo = ck[c][2]
            lhsT = g_tiles[c][:, (k - g_lo) * OUT_DIM : (k - g_lo + 1) * OUT_DIM]

            r = rpool.tile([IN_DIM, BS], F32, tag="r")
            if _dve_k(k):
                nc.vector.tensor_scalar(
                    r[:], u[:], float(k), 0.0, ALU.subtract, ALU.max
                )
            else:
                nc.scalar.activation(
                    r[:], u[:], ACTF.Relu, bias=nk[:, k - 1 : k], scale=1.0
                )

            nc.tensor.matmul(yp[:], lhsT, r[:], start=False, stop=(k == NK))

        # ---- bias add + store ----
        yt = cpool.tile([OUT_DIM, BS], F32, tag="yt")
        nc.vector.tensor_scalar(yt[:], yp[:], bt[:], None, ALU.add)
        nc.sync.dma_start(out=yT[:, :], in_=yt[:])


_NC_CACHE = {}


def _get_program():
    if "nc" not in _NC_CACHE:
        _NC_CACHE["nc"] = build_program()
    return _NC_CACHE["nc"]


def make_in_maps(x, coeff, bias):
    x = np.ascontiguousarray(np.asarray(x, dtype=np.float32))
    coeff_r = np.ascontiguousarray(
        np.asarray(coeff, dtype=np.float32).reshape(IN_DIM, GRID * OUT_DIM)
    )
    bias_r = np.ascontiguousarray(
        np.asarray(bias, dtype=np.float32).reshape(OUT_DIM, 1)
    )
    negk = np.ascontiguousarray(
        np.broadcast_to(
            -np.arange(1, NK + 1, dtype=np.float32)[None, :], (IN_DIM, NK)
        ).copy()
    )
    in_maps = []
    for c in range(N_CORES):
        xs = np.ascontiguousarray(x[c * BS : (c + 1) * BS, :].T)
        in_maps.append({"xT": xs, "coeff": coeff_r, "bias": bias_r, "negk": negk})
    return in_maps


def kernel(x, coeff, bias):
    nc = _get_program()
    in_maps = make_in_maps(x, coeff, bias)
    res = run_bass_kernel_spmd(nc, in_maps, list(range(N_CORES)))
    y = np.concatenate([r["yT"].T for r in res.results], axis=0)
    return np.ascontiguousarray(y.astype(np.float32))


if __name__ == "__main__":
    xx = np.random.randn(B, IN_DIM).astype(np.float32)
    cc = (np.random.randn(IN_DIM, GRID, OUT_DIM) * 0.02).astype(np.float32)
    bb = np.zeros(OUT_DIM, dtype=np.float32)
    yy = kernel(xx, cc, bb)
    print("kernel output:", yy.shape, yy.dtype, float(np.abs(yy).mean()))
